# revision 3
# baseline (speedup 1.0000x reference)
"""CQAttention (BiDAF-style context-query attention) Trainium2 kernel.

Data-parallel over batch: 32 batches -> 8 cores x 4 batches.

Math (per batch, d=128, Lc=2048, Lq=512):
  S = s0[c] + s1[q] + s2[c,q] + bias,  s2 = (Ct*w_mul) @ Qt^T
  S1 = softmax_q(S + NEG*(1-qm));  S2 = softmax_c(S + NEG*(1-cm))
  A  = S1 @ Qt;  Bm = S1 @ (S2^T @ Ct)
  out = [Ct; A; Ct*A; Ct*Bm]^T  -> [4d, Lc]

Device algebra (two score layouts, both needed since the PE contracts only
over partitions):
  X2[c,q]  = exp(s2)            (plain; g folded into consumers)
  X1h[q,c] = exp(s2^T + lnh[q]) (ACT per-partition bias, lnh = s1 + qneg)
  cs[q] = sum_c g[c] X2[c,q]    (N=1 column matmuls, lhsT = X2 blocks,
                                 rhs = g column; out-free=1 is ~free in PE)
  rs[c] = sum_q X1h[q,c]        (N=1 column matmuls, rhs = ones column)
  NUg[d,q] = sum_c (Ct*g)[c,d] X2[c,q]   (lhsT = host-folded Ctg)
  ucs[q,d] = NUg^T / cs         (PE transpose + per-partition scalar mul)
  An[d,c] = sum_q Qt[q,d] X1h[q,c];  Bn[d,c] = sum_q ucs[q,d] X1h[q,c]
  A = An/rs, Bm = Bn/rs  (rs replicated across partitions via transpose +
                          DRAM rearrange + gpsimd partition_broadcast)
  Host assembles output block 0 (= C) directly; device emits A/CtA/CtBm.
"""

import sys

sys.path.insert(0, "/opt/trn_rl_repo")

import numpy as np
from contextlib import ExitStack

NEG = -1e30
N_CORES = 8
B_LOC = 4  # batches per core
D = 128
LC = 2048
LQ = 512
NQT = LQ // 128  # 4 q tiles
NCT = LC // 128  # 16 c tiles

_NC_CACHE = {}


def _build_bass():
    import concourse.bass as bass
    import concourse.bacc as bacc
    import concourse.tile as tile
    from concourse import mybir

    f32 = mybir.dt.float32
    bf16 = mybir.dt.bfloat16
    Exp = mybir.ActivationFunctionType.Exp
    Alu = mybir.AluOpType

    nc = bacc.Bacc("TRN2", target_bir_lowering=False, debug=False)

    Cbf_in = nc.dram_tensor("Cbf", [B_LOC, 128, LC], bf16, kind="ExternalInput").ap()
    Ctg_in = nc.dram_tensor("Ctg", [B_LOC, 128, LC], bf16, kind="ExternalInput").ap()
    Qw_in = nc.dram_tensor("Qw", [B_LOC, 128, LQ], bf16, kind="ExternalInput").ap()
    Qtb_in = nc.dram_tensor("Qtb", [B_LOC, 128, LQ], bf16, kind="ExternalInput").ap()
    Gcol_in = nc.dram_tensor("Gcol", [B_LOC, 128, NCT], bf16, kind="ExternalInput").ap()
    Lnh_in = nc.dram_tensor("Lnh", [B_LOC, 128, NQT], f32, kind="ExternalInput").ap()
    Ident_in = nc.dram_tensor("Ident", [128, 128], f32, kind="ExternalInput").ap()
    Out = nc.dram_tensor("out", [B_LOC, 3 * D, LC], f32, kind="ExternalOutput").ap()
    Rrow = nc.dram_tensor("rrow_scratch", [B_LOC, 1, LC], bf16).ap()

    with tile.TileContext(nc) as tc, ExitStack() as ctx:
        cpool = ctx.enter_context(tc.tile_pool(name="const", bufs=1))
        inp = ctx.enter_context(tc.tile_pool(name="inp", bufs=2))
        epool = ctx.enter_context(tc.tile_pool(name="epool", bufs=2))
        work = ctx.enter_context(tc.tile_pool(name="work", bufs=2))
        opool = ctx.enter_context(tc.tile_pool(name="ostg", bufs=2))
        pp = ctx.enter_context(tc.tile_pool(name="pp", bufs=2, space="PSUM"))

        identf = cpool.tile([128, 128], f32, tag="identf")
        nc.sync.dma_start(identf[:], Ident_in)
        onescol = cpool.tile([128, 1], bf16, tag="ones")
        nc.vector.memset(onescol[:], 1.0)

        def emit_load(b):
            st = {}
            st["cbf"] = inp.tile([128, LC], bf16, tag="cbf", name="cbf")
            nc.sync.dma_start(st["cbf"][:], Cbf_in[b])
            st["ctg"] = inp.tile([128, LC], bf16, tag="ctg", name="ctg")
            nc.sync.dma_start(st["ctg"][:], Ctg_in[b])
            st["qw"] = inp.tile([128, LQ], bf16, tag="qw", name="qw")
            nc.sync.dma_start(st["qw"][:], Qw_in[b])
            st["qtb"] = inp.tile([128, LQ], bf16, tag="qtb", name="qtb")
            nc.sync.dma_start(st["qtb"][:], Qtb_in[b])
            st["gcol"] = inp.tile([128, NCT], bf16, tag="gcol", name="gcol")
            nc.sync.dma_start(st["gcol"][:], Gcol_in[b])
            st["lnh"] = inp.tile([128, NQT], f32, tag="lnh", name="lnh")
            nc.sync.dma_start(st["lnh"][:], Lnh_in[b])
            st["e2"] = []
            st["x1"] = []
            return st

        def emit_score_pair(b, st, step):
            # cq layout: X2 group of 4 c-tiles [128(c), 4*512(q)]
            wA = pp.tile([128, 2048], f32, tag="big")
            for j in range(4):
                ct = step * 4 + j
                nc.tensor.matmul(
                    wA[:, j * 512:(j + 1) * 512],
                    st["cbf"][:, ct * 128:(ct + 1) * 128],
                    st["qw"][:],
                    start=True, stop=True,
                )
            e2 = epool.tile([128, 2048], bf16, tag=f"e2_{step}", name=f"e2_{step}")
            nc.scalar.activation(e2[:], wA[:], Exp)
            st["e2"].append(e2)
            # qc layout: X1h q-tile [128(q), 2048(c)], bias = lnh[q]
            wB = pp.tile([128, 2048], f32, tag="big")
            for cc in range(4):
                nc.tensor.matmul(
                    wB[:, cc * 512:(cc + 1) * 512],
                    st["qw"][:, step * 128:(step + 1) * 128],
                    st["cbf"][:, cc * 512:(cc + 1) * 512],
                    start=True, stop=True,
                )
            x1 = epool.tile([128, 2048], bf16, tag=f"x1_{step}", name=f"x1_{step}")
            nc.scalar.activation(x1[:], wB[:], Exp, bias=st["lnh"][:, step:step + 1])
            st["x1"].append(x1)

        def emit_rs_chain(b, st):
            # rs[c] column sums (N=1 matmuls), then reciprocal and the
            # replication chain: transpose -> DRAM rearrange -> gpsimd bcast
            trs = pp.tile([128, 2048], f32, tag="big")
            for cb in range(NCT):
                for qt in range(NQT):
                    nc.tensor.matmul(
                        trs[:, cb:cb + 1],
                        st["x1"][qt][:, cb * 128:(cb + 1) * 128],
                        onescol[:],
                        start=(qt == 0), stop=(qt == NQT - 1),
                    )
            rrec_col = work.tile([128, NCT], f32, tag="rreccol")
            nc.vector.reciprocal(rrec_col[:], trs[:, 0:NCT])
            nc.tensor.transpose(trs[0:16, 512:640], rrec_col[:], identf[:])
            rrowb = work.tile([16, 128], bf16, tag="rrowb")
            nc.vector.tensor_copy(rrowb[:], trs[0:16, 512:640])
            nc.sync.dma_start(
                Rrow[b, 0].rearrange("(p f) -> p f", p=16, f=128), rrowb[:]
            )
            rrow1 = work.tile([1, LC], bf16, tag="rrow1")
            nc.sync.dma_start(rrow1[:], Rrow[b])
            rrec_rep = work.tile([128, LC], bf16, tag="rrecrep")
            nc.gpsimd.partition_broadcast(rrec_rep[:], rrow1[:])
            st["rrec_rep"] = rrec_rep

        def emit_nug(b, st):
            tnug = pp.tile([128, 2048], f32, tag="big")
            # cs[q] column sums into spare columns of the NUg tile
            for qb in range(NQT):
                for ct in range(NCT):
                    nc.tensor.matmul(
                        tnug[:, 1024 + qb:1025 + qb],
                        st["e2"][ct // 4][:, (ct % 4) * 512 + qb * 128:(ct % 4) * 512 + (qb + 1) * 128],
                        st["gcol"][:, ct:ct + 1],
                        start=(ct == 0), stop=(ct == NCT - 1),
                    )
            for ct in range(NCT):
                nc.tensor.matmul(
                    tnug[:, 0:512],
                    st["ctg"][:, ct * 128:(ct + 1) * 128],
                    st["e2"][ct // 4][:, (ct % 4) * 512:(ct % 4 + 1) * 512],
                    start=(ct == 0), stop=(ct == NCT - 1),
                )
            csr = work.tile([128, NQT], f32, tag="csr")
            nc.vector.reciprocal(csr[:], tnug[:, 1024:1024 + NQT])
            utb = work.tile([128, 512], f32, tag="utb")
            nc.vector.tensor_copy(utb[:], tnug[:, 0:512])
            for qt in range(NQT):
                nc.tensor.transpose(
                    tnug[:, 512 + qt * 128:512 + (qt + 1) * 128],
                    utb[:, qt * 128:(qt + 1) * 128],
                    identf[:],
                )
            uch = work.tile([128, 512], bf16, tag="uch")
            for qt in range(NQT):
                nc.vector.tensor_scalar_mul(
                    uch[:, qt * 128:(qt + 1) * 128],
                    tnug[:, 512 + qt * 128:512 + (qt + 1) * 128],
                    csr[:, qt:qt + 1],
                )
            st["uch"] = uch

        def emit_an(b, st):
            tan = pp.tile([128, 2048], f32, tag="big")
            for cc in range(4):
                for qt in range(NQT):
                    nc.tensor.matmul(
                        tan[:, cc * 512:(cc + 1) * 512],
                        st["qtb"][:, qt * 128:(qt + 1) * 128],
                        st["x1"][qt][:, cc * 512:(cc + 1) * 512],
                        start=(qt == 0), stop=(qt == NQT - 1),
                    )
            a_full = opool.tile([128, LC], f32, tag="a")
            nc.vector.scalar_tensor_tensor(
                a_full[:], tan[:], 0.0, st["rrec_rep"][:],
                op0=Alu.bypass, op1=Alu.mult,
            )
            cta = opool.tile([128, LC], f32, tag="cta")
            nc.gpsimd.tensor_mul(cta[:], st["cbf"][:], a_full[:])
            nc.sync.dma_start(Out[b, 0:128, :], a_full[:])
            nc.sync.dma_start(Out[b, 128:256, :], cta[:])

        def emit_bn(b, st):
            tbn = pp.tile([128, 2048], f32, tag="big")
            for cc in range(4):
                for qt in range(NQT):
                    nc.tensor.matmul(
                        tbn[:, cc * 512:(cc + 1) * 512],
                        st["uch"][:, qt * 128:(qt + 1) * 128],
                        st["x1"][qt][:, cc * 512:(cc + 1) * 512],
                        start=(qt == 0), stop=(qt == NQT - 1),
                    )
            bm_full = opool.tile([128, LC], f32, tag="bm")
            nc.vector.scalar_tensor_tensor(
                bm_full[:], tbn[:], 0.0, st["rrec_rep"][:],
                op0=Alu.bypass, op1=Alu.mult,
            )
            ctb = opool.tile([128, LC], f32, tag="ctb")
            nc.gpsimd.tensor_mul(ctb[:], st["cbf"][:], bm_full[:])
            nc.sync.dma_start(Out[b, 256:384, :], ctb[:])

        # Software-pipelined: batch b's scores interleave with batch b-1's
        # matmul tail so the PE never waits on the ACT exp stream.
        states = {}
        prev = None
        for b in range(B_LOC):
            states[b] = emit_load(b)
            st = states[b]
            if prev is not None:
                emit_rs_chain(prev, states[prev])
            emit_score_pair(b, st, 0)
            if prev is not None:
                emit_nug(prev, states[prev])
            emit_score_pair(b, st, 1)
            emit_score_pair(b, st, 2)
            if prev is not None:
                emit_an(prev, states[prev])
                emit_bn(prev, states[prev])
                del states[prev]
            emit_score_pair(b, st, 3)
            prev = b
        emit_rs_chain(prev, states[prev])
        emit_nug(prev, states[prev])
        emit_an(prev, states[prev])
        emit_bn(prev, states[prev])

    nc.compile()
    return nc


def _prep_inputs(C, Q, Cmask, Qmask, w_c, w_q, w_mul, bias):
    """Host-side precompute of folded factors; returns per-core in_maps."""
    import ml_dtypes

    C = np.asarray(C, dtype=np.float32)
    Q = np.asarray(Q, dtype=np.float32)
    cm = np.asarray(Cmask, dtype=np.float32)
    qm = np.asarray(Qmask, dtype=np.float32)
    w_c = np.asarray(w_c, dtype=np.float32).reshape(D)
    w_q = np.asarray(w_q, dtype=np.float32).reshape(D)
    w_mul = np.asarray(w_mul, dtype=np.float32).reshape(D)

    s0 = np.einsum("bdc,d->bc", C, w_c)  # [B, Lc]
    s1 = np.einsum("bdq,d->bq", Q, w_q)  # [B, Lq]
    g = np.exp(np.where(cm > 0, s0, NEG))  # [B, Lc]; masked c -> exactly 0
    lnh = np.where(qm > 0, s1, NEG)  # [B, Lq]; ACT bias, exp -> exactly 0

    Qw = Q * w_mul[None, :, None]  # [B, d, Lq]
    bf = ml_dtypes.bfloat16
    ident = np.eye(128, dtype=np.float32)

    in_maps = []
    for core in range(N_CORES):
        sl = slice(core * B_LOC, (core + 1) * B_LOC)
        gb = g[sl]  # [4, Lc]
        # Ctg[b, p, ct*128+dd] = C[b, dd, ct*128+p] * g[b, ct*128+p]
        Ctgl = (C[sl] * gb[:, None, :]).reshape(B_LOC, D, NCT, 128)
        Ctgl = Ctgl.transpose(0, 3, 2, 1).reshape(B_LOC, 128, LC)
        # Qtb[b, p, qt*128+dd] = Q[b, dd, qt*128+p]
        qtb = Q[sl].reshape(B_LOC, D, NQT, 128).transpose(0, 3, 2, 1).reshape(B_LOC, 128, LQ)
        gcol = gb.reshape(B_LOC, NCT, 128).transpose(0, 2, 1)  # [4,128,16]
        lnhcol = lnh[sl].reshape(B_LOC, NQT, 128).transpose(0, 2, 1)  # [4,128,4]
        in_maps.append({
            "Cbf": np.ascontiguousarray(C[sl]).astype(bf),
            "Ctg": np.ascontiguousarray(Ctgl).astype(bf),
            "Qw": np.ascontiguousarray(Qw[sl]).astype(bf),
            "Qtb": np.ascontiguousarray(qtb).astype(bf),
            "Gcol": np.ascontiguousarray(gcol).astype(bf),
            "Lnh": np.ascontiguousarray(lnhcol),
            "Ident": ident,
        })
    return in_maps


def kernel(C, Q, Cmask, Qmask, w_c, w_q, w_mul, bias):
    from concourse.bass_utils import run_bass_kernel_spmd

    if "nc" not in _NC_CACHE:
        _NC_CACHE["nc"] = _build_bass()
    nc = _NC_CACHE["nc"]

    in_maps = _prep_inputs(C, Q, Cmask, Qmask, w_c, w_q, w_mul, bias)
    res = run_bass_kernel_spmd(nc, in_maps, list(range(N_CORES)))
    dev = np.concatenate(
        [res.results[i]["out"] for i in range(N_CORES)], axis=0
    ).astype(np.float32)  # [B, 384, Lc] = A / CtA / CtBm
    B = dev.shape[0]
    out = np.empty((B, 4 * D, LC), dtype=np.float32)
    out[:, 0:D, :] = np.asarray(C, dtype=np.float32)  # block Ct = C exactly
    out[:, D:, :] = dev
    return out
